# revision 11
# baseline (speedup 1.0000x reference)
"""Trainium2 Bass kernel for ColumnAttention:
    out = softmax(query @ x^T + bias) @ x        (per batch sample)

Shapes: x [64, 576, 1024] f32, query [576, 1024] f32, bias [576, 576] f32.
Data-parallel over batch across 8 NeuronCores (8 samples per core).

Per-core program (bf16 matmul inputs, fp32 PSUM accumulate):
  Samples are processed in PAIRS: the pair's key axis is 2*576 = 1152 =
  9*128, so every mm1 k-chunk has full 128 partitions (no ragged tails).

  mm1:  scoresT[k, q] = sum_d x[k, d] * qT[d, q]     (k = pair key axis)
        - lhsT = host-pretransposed x (d on partitions)
        - rhs  = host-pretransposed query, q split 288+288 into two PSUM
          banks of one 2-bank tile (cols 0:288 and 512:800) so every
          matmul has N=288 (no N=64 tail instructions)
  bias: DVE adds host-pretransposed [biasT; biasT] during PSUM->SBUF drain
  exp:  ACT exp (scores are O(+-6): no max subtraction needed), bf16 out
  mm2:  out[q, d] = sum_k attnT[k, q]^T * x[k, d]    (per sample, 5 k-steps)
        - attnT from exp is directly the stationary operand (no transpose)
        - rhs = x natural; an N=1 ones-column matmul accumulates the
          softmax denominator
  norm: DVE reciprocal; ACT Copy with per-partition scale on PSUM drain.

  mm1 of pair p+1 is interleaved chunk-wise between mm2 steps of pair p,
  so each PSUM pool's drain latency hides under the other matmul stream
  (psO runs single-buffered; total PSUM = 4+2+1 = 7 banks).
"""

import sys

if "/opt/trn_rl_repo" not in sys.path:
    sys.path.insert(0, "/opt/trn_rl_repo")

import numpy as np
import ml_dtypes
from contextlib import ExitStack

B, NQ, D = 64, 576, 1024
NCORES = 8
BPC = B // NCORES      # samples per core
NPAIR = BPC // 2       # sample pairs per core

P = 128
NKC = 2 * NQ // P      # 9 pair k-chunks
NDC = D // P           # 8 d chunks
QCH = [(i * P, min(P, NQ - i * P)) for i in range((NQ + P - 1) // P)]  # q chunks

_BUILD_CACHE = {}


def build_program():
    """Build + compile the per-core Bass program. Returns the Bacc object."""
    if "nc" in _BUILD_CACHE:
        return _BUILD_CACHE["nc"]

    import concourse.mybir as mybir
    import concourse.tile as tile
    from concourse import bacc

    bf16 = mybir.dt.bfloat16
    f32 = mybir.dt.float32
    AF = mybir.ActivationFunctionType

    nc = bacc.Bacc(trn_type="TRN2", target_bir_lowering=False, debug=False)

    xs = nc.dram_tensor("xs", [BPC, NQ, D], bf16, kind="ExternalInput")
    xsT = nc.dram_tensor("xsT", [BPC, D, NQ], bf16, kind="ExternalInput")
    qT = nc.dram_tensor("qT", [D, NQ], bf16, kind="ExternalInput")
    bTp = nc.dram_tensor("bTp", [2 * NQ, NQ], bf16, kind="ExternalInput")
    out = nc.dram_tensor("out", [BPC, NQ, D], f32, kind="ExternalOutput")

    with tile.TileContext(nc) as tc, ExitStack() as ctx:
        statics = ctx.enter_context(tc.tile_pool(name="statics", bufs=1))
        xpool = ctx.enter_context(tc.tile_pool(name="xpool", bufs=2))
        xtpool = ctx.enter_context(tc.tile_pool(name="xtpool", bufs=2))
        scpool = ctx.enter_context(tc.tile_pool(name="scpool", bufs=3))
        atpool = ctx.enter_context(tc.tile_pool(name="atpool", bufs=2))
        opool = ctx.enter_context(tc.tile_pool(name="opool", bufs=3))
        rpool = ctx.enter_context(tc.tile_pool(name="rpool", bufs=3))
        # PSUM: 3 + 4 + 1 = 8 banks
        psAB = ctx.enter_context(tc.tile_pool(name="psAB", bufs=3, space="PSUM"))
        psO = ctx.enter_context(tc.tile_pool(name="psO", bufs=2, space="PSUM"))
        psS = ctx.enter_context(tc.tile_pool(name="psS", bufs=1, space="PSUM"))

        # ---- static params (qT first: it gates the first matmul) ----
        qT_sb = statics.tile([P, NDC, NQ], bf16)
        qT_r = qT.ap().rearrange("(c p) q -> p c q", p=P)
        nc.gpsimd.dma_start(out=qT_sb[:, 0:4, :], in_=qT_r[:, 0:4, :])
        nc.sync.dma_start(out=qT_sb[:, 4:8, :], in_=qT_r[:, 4:8, :])
        bT_sb = statics.tile([P, NKC, NQ], bf16)
        ones_sb = statics.tile([P, 1], bf16)
        nc.vector.memset(ones_sb, 1.0)

        def load_pair(pr):
            """DMA pair pr's x (natural, pair-k layout) and xT.
            Big transfers are split across several queues for parallelism."""
            x_sb = xpool.tile([P, NKC, D], bf16, tag="x")
            x_r = (xs.ap()[2 * pr:2 * pr + 2].rearrange("b n d -> (b n) d")
                   .rearrange("(c p) d -> p c d", p=P))
            for lo, hi in ((0, 3), (3, 6), (6, 9)):
                nc.gpsimd.dma_start(out=x_sb[:, lo:hi, :], in_=x_r[:, lo:hi, :])
            xT_sb = xtpool.tile([P, NDC, 2 * NQ], bf16, tag="xT")
            for s in range(2):
                xT_r = xsT.ap()[2 * pr + s].rearrange("(c p) k -> p c k", p=P)
                nc.sync.dma_start(
                    out=xT_sb[:, 0:4, s * NQ:(s + 1) * NQ], in_=xT_r[:, 0:4, :])
                nc.scalar.dma_start(
                    out=xT_sb[:, 4:8, s * NQ:(s + 1) * NQ], in_=xT_r[:, 4:8, :])
            return x_sb, xT_sb

        def mm1_chunk(xT_sb, attnT, kc):
            """One pair k-chunk of scoresT + bias + exp."""
            pa1 = psAB.tile([P, 512], mybir.dt.float32, tag="pa")
            pa2 = psAB.tile([P, 512], mybir.dt.float32, tag="pa")
            for dc in range(NDC):
                w = xT_sb[:, dc, kc * P:(kc + 1) * P]
                st, sp = dc == 0, dc == NDC - 1
                nc.tensor.matmul(pa1[:, 0:288], w, qT_sb[:, dc, 0:288], start=st, stop=sp)
                nc.tensor.matmul(pa2[:, 0:288], w, qT_sb[:, dc, 288:576], start=st, stop=sp)
            sc = scpool.tile([P, NQ], mybir.dt.float32, tag="sc")
            nc.vector.tensor_add(sc[:, 0:288], pa1[:, 0:288], bT_sb[:, kc, 0:288])
            nc.vector.tensor_add(sc[:, 288:576], pa2[:, 0:288], bT_sb[:, kc, 288:576])
            nc.scalar.activation(attnT[:, kc, :], sc, AF.Exp)

        def mm2_step(pr, s, qc, x_sb, attnT, ps_):
            """One (sample, q-chunk) of out = attn @ x, plus denominator.

            Sample order alternates s0/s1 within each q-chunk; s0 ends and s1
            starts on the K=64 straddle chunk so the two half-array matmuls
            sit adjacent in the PE queue (disjoint row groups -> concurrent).
            """
            qb, qs = QCH[qc]
            if s == 0:
                steps = [(c, 0, P) for c in range(4)] + [(4, 0, 64)]
            else:
                steps = [(4, 64, 64)] + [(c, 0, P) for c in range(5, 9)]
            po = psO.tile([P, 1024], mybir.dt.float32, tag="po")
            for j, (c, pb, K) in enumerate(steps):
                w = attnT[pb:pb + K, c, qb:qb + qs]
                st, sp = j == 0, j == len(steps) - 1
                nc.tensor.matmul(po[0:qs, 0:512], w, x_sb[pb:pb + K, c, 0:512], start=st, stop=sp)
                nc.tensor.matmul(po[0:qs, 512:1024], w, x_sb[pb:pb + K, c, 512:1024], start=st, stop=sp)
                nc.tensor.matmul(ps_[0:qs, s:s + 1], w, ones_sb[pb:pb + K, :], start=st, stop=sp)
            r = rpool.tile([P, 1], mybir.dt.float32, tag="r")
            nc.vector.reciprocal(r[0:qs, :], ps_[0:qs, s:s + 1])
            o = opool.tile([P, D], mybir.dt.float32, tag="o")
            nc.scalar.activation(o[0:qs, :], po[0:qs, :], AF.Copy, scale=r[0:qs, :])
            nc.gpsimd.dma_start(out=out.ap()[2 * pr + s, qb:qb + qs, :], in_=o[0:qs, :])

        # ---- prologue: pair 0 (bias load after pair-0 x/xT: off critical path) ----
        x_cur, xT_cur = load_pair(0)
        nc.gpsimd.dma_start(out=bT_sb, in_=bTp.ap().rearrange("(c p) q -> p c q", p=P))
        attnT_cur = atpool.tile([P, NKC, NQ], bf16, tag="attnT")
        for kc in range(NKC):
            mm1_chunk(xT_cur, attnT_cur, kc)

        # ---- steady: mm2(pair p) interleaved with mm1(pair p+1) ----
        for pr in range(NPAIR):
            if pr + 1 < NPAIR:
                x_nxt, xT_nxt = load_pair(pr + 1)
                attnT_nxt = atpool.tile([P, NKC, NQ], bf16, tag="attnT")
            else:
                x_nxt = xT_nxt = attnT_nxt = None
            steps = [(s, qc) for qc in range(len(QCH)) for s in range(2)]
            ps_cur = None
            for i, (s, qc) in enumerate(steps):
                if s == 0:
                    ps_cur = psS.tile([P, 2], mybir.dt.float32, tag="ps")
                mm2_step(pr, s, qc, x_cur, attnT_cur, ps_cur)
                if attnT_nxt is not None and i < NKC:
                    mm1_chunk(xT_nxt, attnT_nxt, i)
            x_cur, xT_cur, attnT_cur = x_nxt, xT_nxt, attnT_nxt

    nc.compile()
    _BUILD_CACHE["nc"] = nc
    return nc


def make_in_maps(x, query, bias):
    qT_np = np.ascontiguousarray(query.T).astype(ml_dtypes.bfloat16)
    bT = np.ascontiguousarray(bias.T).astype(ml_dtypes.bfloat16)
    bTp_np = np.concatenate([bT, bT], axis=0)
    x_bf = x.astype(ml_dtypes.bfloat16)
    xT_bf = np.ascontiguousarray(x_bf.transpose(0, 2, 1))
    in_maps = []
    for c in range(NCORES):
        in_maps.append({
            "xs": np.ascontiguousarray(x_bf[c * BPC:(c + 1) * BPC]),
            "xsT": np.ascontiguousarray(xT_bf[c * BPC:(c + 1) * BPC]),
            "qT": qT_np,
            "bTp": bTp_np,
        })
    return in_maps


def kernel(x, query, bias):
    from concourse.bass_utils import run_bass_kernel_spmd

    nc = build_program()
    in_maps = make_in_maps(np.asarray(x), np.asarray(query), np.asarray(bias))
    res = run_bass_kernel_spmd(nc, in_maps, core_ids=list(range(NCORES)))
    return np.concatenate([r["out"] for r in res.results], axis=0)


if __name__ == "__main__":
    rng = np.random.default_rng(0)
    x = rng.standard_normal((B, NQ, D), dtype=np.float32)
    q = rng.standard_normal((NQ, D), dtype=np.float32) / 32.0
    bias = 0.01 * rng.standard_normal((NQ, NQ), dtype=np.float32)
    o = kernel(x, q, bias)
    print(o.shape, o.dtype)


# revision 13
# speedup vs baseline: 1.0102x; 1.0102x over previous
"""Trainium2 Bass kernel for ColumnAttention:
    out = softmax(query @ x^T + bias) @ x        (per batch sample)

Shapes: x [64, 576, 1024] f32, query [576, 1024] f32, bias [576, 576] f32.
Data-parallel over batch across 8 NeuronCores (8 samples per core).

Per-core program (bf16 matmul inputs, fp32 PSUM accumulate):
  Samples are processed in PAIRS: the pair's key axis is 2*576 = 1152 =
  9*128, so every mm1 k-chunk has full 128 partitions (no ragged tails).

  mm1:  scoresT[k, q] = sum_d x[k, d] * qT[d, q]     (k = pair key axis)
        - lhsT = host-pretransposed x (d on partitions)
        - rhs  = host-pretransposed query, q split 288+288 into two PSUM
          banks of one 2-bank tile (cols 0:288 and 512:800) so every
          matmul has N=288 (no N=64 tail instructions)
  bias: DVE adds host-pretransposed [biasT; biasT] during PSUM->SBUF drain
  exp:  ACT exp (scores are O(+-6): no max subtraction needed), bf16 out
  mm2:  out[q, d] = sum_k attnT[k, q]^T * x[k, d]    (per sample, 5 k-steps)
        - attnT from exp is directly the stationary operand (no transpose)
        - rhs = x natural; an N=1 ones-column matmul accumulates the
          softmax denominator
  norm: DVE reciprocal; ACT Copy with per-partition scale on PSUM drain.

  mm1 of pair p+1 is interleaved chunk-wise between mm2 steps of pair p,
  so each PSUM pool's drain latency hides under the other matmul stream
  (psO runs single-buffered; total PSUM = 4+2+1 = 7 banks).
"""

import sys

if "/opt/trn_rl_repo" not in sys.path:
    sys.path.insert(0, "/opt/trn_rl_repo")

import numpy as np
import ml_dtypes
from contextlib import ExitStack

B, NQ, D = 64, 576, 1024
NCORES = 8
BPC = B // NCORES      # samples per core
NPAIR = BPC // 2       # sample pairs per core

P = 128
NKC = 2 * NQ // P      # 9 pair k-chunks
NDC = D // P           # 8 d chunks
QCH = [(i * P, min(P, NQ - i * P)) for i in range((NQ + P - 1) // P)]  # q chunks

_BUILD_CACHE = {}


def build_program():
    """Build + compile the per-core Bass program. Returns the Bacc object."""
    if "nc" in _BUILD_CACHE:
        return _BUILD_CACHE["nc"]

    import concourse.mybir as mybir
    import concourse.tile as tile
    from concourse import bacc

    bf16 = mybir.dt.bfloat16
    f32 = mybir.dt.float32
    AF = mybir.ActivationFunctionType

    nc = bacc.Bacc(trn_type="TRN2", target_bir_lowering=False, debug=False)

    xs = nc.dram_tensor("xs", [BPC, NQ, D], bf16, kind="ExternalInput")
    xsT = nc.dram_tensor("xsT", [BPC, D, NQ], bf16, kind="ExternalInput")
    qT = nc.dram_tensor("qT", [D, NQ], bf16, kind="ExternalInput")
    bTp = nc.dram_tensor("bTp", [2 * NQ, NQ], bf16, kind="ExternalInput")
    out = nc.dram_tensor("out", [BPC, NQ, D], f32, kind="ExternalOutput")

    with tile.TileContext(nc) as tc, ExitStack() as ctx:
        statics = ctx.enter_context(tc.tile_pool(name="statics", bufs=1))
        xpool = ctx.enter_context(tc.tile_pool(name="xpool", bufs=2))
        xtpool = ctx.enter_context(tc.tile_pool(name="xtpool", bufs=2))
        scpool = ctx.enter_context(tc.tile_pool(name="scpool", bufs=3))
        atpool = ctx.enter_context(tc.tile_pool(name="atpool", bufs=2))
        opool = ctx.enter_context(tc.tile_pool(name="opool", bufs=3))
        rpool = ctx.enter_context(tc.tile_pool(name="rpool", bufs=3))
        # PSUM: 3 + 4 + 1 = 8 banks
        psAB = ctx.enter_context(tc.tile_pool(name="psAB", bufs=3, space="PSUM"))
        psO = ctx.enter_context(tc.tile_pool(name="psO", bufs=2, space="PSUM"))
        psS = ctx.enter_context(tc.tile_pool(name="psS", bufs=1, space="PSUM"))

        # ---- static params (qT first, dc-progressive: mm1 consumes slices
        # in dc order, so matmuls start after the first slice lands) ----
        qT_sb = statics.tile([P, NDC, NQ], bf16)
        qT_r = qT.ap().rearrange("(c p) q -> p c q", p=P)
        for dc in range(NDC):
            nc.gpsimd.dma_start(out=qT_sb[:, dc, :], in_=qT_r[:, dc, :])
        bT_sb = statics.tile([P, NKC, NQ], bf16)
        ones_sb = statics.tile([P, 1], bf16)
        nc.vector.memset(ones_sb, 1.0)

        def load_pair(pr):
            """DMA pair pr's x (natural, pair-k layout) and xT.
            xT loads are k-progressive (mm1 consumes k-chunks in order);
            big transfers are split across queues for parallelism."""
            x_sb = xpool.tile([P, NKC, D], bf16, tag="x")
            x_r = (xs.ap()[2 * pr:2 * pr + 2].rearrange("b n d -> (b n) d")
                   .rearrange("(c p) d -> p c d", p=P))
            for lo, hi in ((0, 3), (3, 6), (6, 9)):
                nc.gpsimd.dma_start(out=x_sb[:, lo:hi, :], in_=x_r[:, lo:hi, :])
            xT_sb = xtpool.tile([P, NDC, 2 * NQ], bf16, tag="xT")
            for s in range(2):
                xT_r = xsT.ap()[2 * pr + s].rearrange("(c p) k -> p c k", p=P)
                for klo, khi in ((0, 288), (288, 576)):
                    nc.sync.dma_start(
                        out=xT_sb[:, :, s * NQ + klo:s * NQ + khi],
                        in_=xT_r[:, :, klo:khi])
            return x_sb, xT_sb

        def mm1_chunk(xT_sb, attnT, kc):
            """One pair k-chunk of scoresT + bias + exp."""
            pa1 = psAB.tile([P, 512], mybir.dt.float32, tag="pa")
            pa2 = psAB.tile([P, 512], mybir.dt.float32, tag="pa")
            for dc in range(NDC):
                w = xT_sb[:, dc, kc * P:(kc + 1) * P]
                st, sp = dc == 0, dc == NDC - 1
                nc.tensor.matmul(pa1[:, 0:288], w, qT_sb[:, dc, 0:288], start=st, stop=sp)
                nc.tensor.matmul(pa2[:, 0:288], w, qT_sb[:, dc, 288:576], start=st, stop=sp)
            sc = scpool.tile([P, NQ], mybir.dt.float32, tag="sc")
            nc.vector.tensor_add(sc[:, 0:288], pa1[:, 0:288], bT_sb[:, kc, 0:288])
            nc.vector.tensor_add(sc[:, 288:576], pa2[:, 0:288], bT_sb[:, kc, 288:576])
            nc.scalar.activation(attnT[:, kc, :], sc, AF.Exp)

        def mm2_step(pr, s, qc, x_sb, attnT, ps_):
            """One (sample, q-chunk) of out = attn @ x, plus denominator.

            Sample order alternates s0/s1 within each q-chunk; s0 ends and s1
            starts on the K=64 straddle chunk so the two half-array matmuls
            sit adjacent in the PE queue (disjoint row groups -> concurrent).
            """
            qb, qs = QCH[qc]
            if s == 0:
                steps = [(c, 0, P) for c in range(4)] + [(4, 0, 64)]
            else:
                steps = [(4, 64, 64)] + [(c, 0, P) for c in range(5, 9)]
            po = psO.tile([P, 1024], mybir.dt.float32, tag="po")
            for j, (c, pb, K) in enumerate(steps):
                w = attnT[pb:pb + K, c, qb:qb + qs]
                st, sp = j == 0, j == len(steps) - 1
                nc.tensor.matmul(po[0:qs, 0:512], w, x_sb[pb:pb + K, c, 0:512], start=st, stop=sp)
                nc.tensor.matmul(po[0:qs, 512:1024], w, x_sb[pb:pb + K, c, 512:1024], start=st, stop=sp)
                nc.tensor.matmul(ps_[0:qs, s:s + 1], w, ones_sb[pb:pb + K, :], start=st, stop=sp)
            r = rpool.tile([P, 1], mybir.dt.float32, tag="r")
            nc.vector.reciprocal(r[0:qs, :], ps_[0:qs, s:s + 1])
            o = opool.tile([P, D], mybir.dt.float32, tag="o")
            nc.scalar.activation(o[0:qs, :], po[0:qs, :], AF.Copy, scale=r[0:qs, :])
            nc.gpsimd.dma_start(out=out.ap()[2 * pr + s, qb:qb + qs, :], in_=o[0:qs, :])

        # ---- prologue: pair 0 (bias load off the critical path, on ACT's queue) ----
        x_cur, xT_cur = load_pair(0)
        nc.scalar.dma_start(out=bT_sb, in_=bTp.ap().rearrange("(c p) q -> p c q", p=P))
        attnT_cur = atpool.tile([P, NKC, NQ], bf16, tag="attnT")
        for kc in range(NKC):
            mm1_chunk(xT_cur, attnT_cur, kc)

        # ---- steady: mm2(pair p) interleaved with mm1(pair p+1) ----
        for pr in range(NPAIR):
            if pr + 1 < NPAIR:
                x_nxt, xT_nxt = load_pair(pr + 1)
                attnT_nxt = atpool.tile([P, NKC, NQ], bf16, tag="attnT")
            else:
                x_nxt = xT_nxt = attnT_nxt = None
            steps = [(s, qc) for qc in range(len(QCH)) for s in range(2)]
            ps_cur = None
            for i, (s, qc) in enumerate(steps):
                if s == 0:
                    ps_cur = psS.tile([P, 2], mybir.dt.float32, tag="ps")
                mm2_step(pr, s, qc, x_cur, attnT_cur, ps_cur)
                if attnT_nxt is not None and i < NKC:
                    mm1_chunk(xT_nxt, attnT_nxt, i)
            x_cur, xT_cur, attnT_cur = x_nxt, xT_nxt, attnT_nxt

    nc.compile()
    _BUILD_CACHE["nc"] = nc
    return nc


def make_in_maps(x, query, bias):
    qT_np = np.ascontiguousarray(query.T).astype(ml_dtypes.bfloat16)
    bT = np.ascontiguousarray(bias.T).astype(ml_dtypes.bfloat16)
    bTp_np = np.concatenate([bT, bT], axis=0)
    x_bf = x.astype(ml_dtypes.bfloat16)
    xT_bf = np.ascontiguousarray(x_bf.transpose(0, 2, 1))
    in_maps = []
    for c in range(NCORES):
        in_maps.append({
            "xs": np.ascontiguousarray(x_bf[c * BPC:(c + 1) * BPC]),
            "xsT": np.ascontiguousarray(xT_bf[c * BPC:(c + 1) * BPC]),
            "qT": qT_np,
            "bTp": bTp_np,
        })
    return in_maps


def kernel(x, query, bias):
    from concourse.bass_utils import run_bass_kernel_spmd

    nc = build_program()
    in_maps = make_in_maps(np.asarray(x), np.asarray(query), np.asarray(bias))
    res = run_bass_kernel_spmd(nc, in_maps, core_ids=list(range(NCORES)))
    return np.concatenate([r["out"] for r in res.results], axis=0)


if __name__ == "__main__":
    rng = np.random.default_rng(0)
    x = rng.standard_normal((B, NQ, D), dtype=np.float32)
    q = rng.standard_normal((NQ, D), dtype=np.float32) / 32.0
    bias = 0.01 * rng.standard_normal((NQ, NQ), dtype=np.float32)
    o = kernel(x, q, bias)
    print(o.shape, o.dtype)


# revision 16
# speedup vs baseline: 1.0335x; 1.0231x over previous
"""Trainium2 Bass kernel for ColumnAttention:
    out = softmax(query @ x^T + bias) @ x        (per batch sample)

Shapes: x [64, 576, 1024] f32, query [576, 1024] f32, bias [576, 576] f32.
Data-parallel over batch across 8 NeuronCores (8 samples per core).

Per-core program (bf16 matmul inputs, fp32 PSUM accumulate):
  Samples are processed in PAIRS: the pair's key axis is 2*576 = 1152 =
  9*128, so every mm1 k-chunk has full 128 partitions (no ragged tails).

  mm1:  scoresT[k, q] = sum_d x[k, d] * qT[d, q]     (k = pair key axis)
        - lhsT = host-pretransposed x (d on partitions)
        - rhs  = host-pretransposed query, q split 288+288 into two PSUM
          banks of one 2-bank tile (cols 0:288 and 512:800) so every
          matmul has N=288 (no N=64 tail instructions)
  bias: DVE adds host-pretransposed [biasT; biasT] during PSUM->SBUF drain
  exp:  ACT exp (scores are O(+-6): no max subtraction needed), bf16 out
  mm2:  out[q, d] = sum_k attnT[k, q]^T * x[k, d]    (per sample, 5 k-steps)
        - attnT from exp is directly the stationary operand (no transpose)
        - rhs = x natural; an N=1 ones-column matmul accumulates the
          softmax denominator
  norm: DVE reciprocal; ACT Copy with per-partition scale on PSUM drain.

  mm1 of pair p+1 is interleaved chunk-wise between mm2 steps of pair p,
  so each PSUM pool's drain latency hides under the other matmul stream
  (psO runs single-buffered; total PSUM = 4+2+1 = 7 banks).
"""

import sys

if "/opt/trn_rl_repo" not in sys.path:
    sys.path.insert(0, "/opt/trn_rl_repo")

import numpy as np
import ml_dtypes
from contextlib import ExitStack

B, NQ, D = 64, 576, 1024
NCORES = 8
BPC = B // NCORES      # samples per core
NPAIR = BPC // 2       # sample pairs per core

P = 128
NKC = 2 * NQ // P      # 9 pair k-chunks
NDC = D // P           # 8 d chunks
QCH = [(i * P, min(P, NQ - i * P)) for i in range((NQ + P - 1) // P)]  # q chunks

_BUILD_CACHE = {}


def build_program():
    """Build + compile the per-core Bass program. Returns the Bacc object."""
    if "nc" in _BUILD_CACHE:
        return _BUILD_CACHE["nc"]

    import concourse.mybir as mybir
    import concourse.tile as tile
    from concourse import bacc

    bf16 = mybir.dt.bfloat16
    f32 = mybir.dt.float32
    AF = mybir.ActivationFunctionType

    nc = bacc.Bacc(trn_type="TRN2", target_bir_lowering=False, debug=False)

    xs = nc.dram_tensor("xs", [BPC, NQ, D], bf16, kind="ExternalInput")
    xsT = nc.dram_tensor("xsT", [BPC, D, NQ], bf16, kind="ExternalInput")
    qT = nc.dram_tensor("qT", [D, NQ], bf16, kind="ExternalInput")
    bTp = nc.dram_tensor("bTp", [2 * NQ, NQ], bf16, kind="ExternalInput")
    out = nc.dram_tensor("out", [BPC, NQ, D], f32, kind="ExternalOutput")

    with tile.TileContext(nc) as tc, ExitStack() as ctx:
        statics = ctx.enter_context(tc.tile_pool(name="statics", bufs=1))
        xpool = ctx.enter_context(tc.tile_pool(name="xpool", bufs=2))
        xtpool = ctx.enter_context(tc.tile_pool(name="xtpool", bufs=2))
        scpool = ctx.enter_context(tc.tile_pool(name="scpool", bufs=3))
        atpool = ctx.enter_context(tc.tile_pool(name="atpool", bufs=2))
        opool = ctx.enter_context(tc.tile_pool(name="opool", bufs=3))
        rpool = ctx.enter_context(tc.tile_pool(name="rpool", bufs=3))
        # PSUM: 2 + 4 + 2 = 8 banks (mm1 chunks are never queue-adjacent —
        # mm2 units alternate between them — so psAB gets by with 2 slots)
        psAB = ctx.enter_context(tc.tile_pool(name="psAB", bufs=2, space="PSUM"))
        psO = ctx.enter_context(tc.tile_pool(name="psO", bufs=2, space="PSUM"))
        psS = ctx.enter_context(tc.tile_pool(name="psS", bufs=2, space="PSUM"))

        # ---- static params (qT first, dc-progressive: mm1 consumes slices
        # in dc order, so matmuls start after the first slice lands) ----
        qT_sb = statics.tile([P, NDC, NQ], bf16)
        qT_r = qT.ap().rearrange("(c p) q -> p c q", p=P)
        for dc in range(NDC):
            nc.gpsimd.dma_start(out=qT_sb[:, dc, :], in_=qT_r[:, dc, :])
        bT_sb = statics.tile([P, NKC, NQ], bf16)
        ones_sb = statics.tile([P, 1], bf16)
        nc.vector.memset(ones_sb, 1.0)

        def load_xT(pr, s, xT_sb):
            xT_r = xsT.ap()[2 * pr + s].rearrange("(c p) k -> p c k", p=P)
            for klo, khi in ((0, 288), (288, 576)):
                nc.sync.dma_start(
                    out=xT_sb[:, :, s * NQ + klo:s * NQ + khi],
                    in_=xT_r[:, :, klo:khi])

        def load_x(pr):
            x_sb = xpool.tile([P, NKC, D], bf16, tag="x")
            x_r = (xs.ap()[2 * pr:2 * pr + 2].rearrange("b n d -> (b n) d")
                   .rearrange("(c p) d -> p c d", p=P))
            for lo, hi in ((0, 3), (3, 6), (6, 9)):
                nc.gpsimd.dma_start(out=x_sb[:, lo:hi, :], in_=x_r[:, lo:hi, :])
            return x_sb

        def load_pair(pr):
            """DMA pair pr's x (natural, pair-k layout) and xT.
            xT loads are k-progressive (mm1 consumes k-chunks in order);
            big transfers are split across queues for parallelism."""
            xT_sb = xtpool.tile([P, NDC, 2 * NQ], bf16, tag="xT")
            load_xT(pr, 0, xT_sb)
            load_xT(pr, 1, xT_sb)
            x_sb = load_x(pr)
            return x_sb, xT_sb

        def mm1_chunk(xT_sb, attnT, kc):
            """One pair k-chunk of scoresT + bias + exp."""
            pa1 = psAB.tile([P, 512], mybir.dt.float32, tag="pa")
            pa2 = psAB.tile([P, 512], mybir.dt.float32, tag="pa")
            for dc in range(NDC):
                w = xT_sb[:, dc, kc * P:(kc + 1) * P]
                st, sp = dc == 0, dc == NDC - 1
                nc.tensor.matmul(pa1[:, 0:288], w, qT_sb[:, dc, 0:288], start=st, stop=sp)
                nc.tensor.matmul(pa2[:, 0:288], w, qT_sb[:, dc, 288:576], start=st, stop=sp)
            sc = scpool.tile([P, NQ], mybir.dt.float32, tag="sc")
            nc.vector.tensor_add(sc[:, 0:288], pa1[:, 0:288], bT_sb[:, kc, 0:288])
            nc.vector.tensor_add(sc[:, 288:576], pa2[:, 0:288], bT_sb[:, kc, 288:576])
            nc.scalar.activation(attnT[:, kc, :], sc, AF.Exp)

        def mm2_step(pr, s, qc, x_sb, attnT, ps_):
            """One (sample, q-chunk) of out = attn @ x, plus denominator.

            Sample order alternates s0/s1 within each q-chunk; s0 ends and s1
            starts on the K=64 straddle chunk so the two half-array matmuls
            sit adjacent in the PE queue (disjoint row groups -> concurrent).
            """
            qb, qs = QCH[qc]
            if s == 0:
                steps = [(c, 0, P) for c in range(4)] + [(4, 0, 64)]
            else:
                steps = [(4, 64, 64)] + [(c, 0, P) for c in range(5, 9)]
            po = psO.tile([P, 1024], mybir.dt.float32, tag="po")
            for j, (c, pb, K) in enumerate(steps):
                w = attnT[pb:pb + K, c, qb:qb + qs]
                st, sp = j == 0, j == len(steps) - 1
                nc.tensor.matmul(po[0:qs, 0:512], w, x_sb[pb:pb + K, c, 0:512], start=st, stop=sp)
                nc.tensor.matmul(po[0:qs, 512:1024], w, x_sb[pb:pb + K, c, 512:1024], start=st, stop=sp)
                nc.tensor.matmul(ps_[0:qs, s:s + 1], w, ones_sb[pb:pb + K, :], start=st, stop=sp)
            r = rpool.tile([P, 1], mybir.dt.float32, tag="r")
            nc.vector.reciprocal(r[0:qs, :], ps_[0:qs, s:s + 1])
            o = opool.tile([P, D], mybir.dt.float32, tag="o")
            nc.scalar.activation(o[0:qs, :], po[0:qs, :], AF.Copy, scale=r[0:qs, :])
            nc.gpsimd.dma_start(out=out.ap()[2 * pr + s, qb:qb + qs, :], in_=o[0:qs, :])

        # ---- prologue: pair 0, loads in consumption-priority order:
        # qT (above) -> xT sample 0 -> bias -> xT sample 1 -> x natural ----
        xT_cur = xtpool.tile([P, NDC, 2 * NQ], bf16, tag="xT")
        load_xT(0, 0, xT_cur)
        nc.scalar.dma_start(out=bT_sb, in_=bTp.ap().rearrange("(c p) q -> p c q", p=P))
        load_xT(0, 1, xT_cur)
        x_cur = load_x(0)
        attnT_cur = atpool.tile([P, NKC, NQ], bf16, tag="attnT")
        for kc in range(NKC):
            mm1_chunk(xT_cur, attnT_cur, kc)

        # ---- steady: mm2(pair p) interleaved with mm1(pair p+1) ----
        for pr in range(NPAIR):
            if pr + 1 < NPAIR:
                x_nxt, xT_nxt = load_pair(pr + 1)
                attnT_nxt = atpool.tile([P, NKC, NQ], bf16, tag="attnT")
            else:
                x_nxt = xT_nxt = attnT_nxt = None
            steps = [(s, qc) for qc in range(len(QCH)) for s in range(2)]
            ps_cur = None
            for i, (s, qc) in enumerate(steps):
                if s == 0:
                    ps_cur = psS.tile([P, 2], mybir.dt.float32, tag="ps")
                mm2_step(pr, s, qc, x_cur, attnT_cur, ps_cur)
                if attnT_nxt is not None and i < NKC:
                    mm1_chunk(xT_nxt, attnT_nxt, i)
            x_cur, xT_cur, attnT_cur = x_nxt, xT_nxt, attnT_nxt

    nc.compile()
    _BUILD_CACHE["nc"] = nc
    return nc


def make_in_maps(x, query, bias):
    qT_np = np.ascontiguousarray(query.T).astype(ml_dtypes.bfloat16)
    bT = np.ascontiguousarray(bias.T).astype(ml_dtypes.bfloat16)
    bTp_np = np.concatenate([bT, bT], axis=0)
    x_bf = x.astype(ml_dtypes.bfloat16)
    xT_bf = np.ascontiguousarray(x_bf.transpose(0, 2, 1))
    in_maps = []
    for c in range(NCORES):
        in_maps.append({
            "xs": np.ascontiguousarray(x_bf[c * BPC:(c + 1) * BPC]),
            "xsT": np.ascontiguousarray(xT_bf[c * BPC:(c + 1) * BPC]),
            "qT": qT_np,
            "bTp": bTp_np,
        })
    return in_maps


def kernel(x, query, bias):
    from concourse.bass_utils import run_bass_kernel_spmd

    nc = build_program()
    in_maps = make_in_maps(np.asarray(x), np.asarray(query), np.asarray(bias))
    res = run_bass_kernel_spmd(nc, in_maps, core_ids=list(range(NCORES)))
    return np.concatenate([r["out"] for r in res.results], axis=0)


if __name__ == "__main__":
    rng = np.random.default_rng(0)
    x = rng.standard_normal((B, NQ, D), dtype=np.float32)
    q = rng.standard_normal((NQ, D), dtype=np.float32) / 32.0
    bias = 0.01 * rng.standard_normal((NQ, NQ), dtype=np.float32)
    o = kernel(x, q, bias)
    print(o.shape, o.dtype)
